# revision 1
# baseline (speedup 1.0000x reference)
import sys

sys.path.insert(0, "/opt/trn_rl_repo")

import numpy as np

# Problem constants (hardcoded per harness contract)
B = 64          # full batch
NC_CORES = 8
BPC = 8         # batches per core
N = 1024
D = 768
NS = 16         # n_slots
KT = 8          # n-tiles of 128
DT = 6          # d-tiles of 128

_CACHE = {}


def _build_nc(debug=False):
    import concourse.bacc as bacc
    import concourse.tile as tile
    import concourse.mybir as mybir
    from concourse.bass import IndirectOffsetOnAxis

    fp32 = mybir.dt.float32
    bf16 = mybir.dt.bfloat16
    i32 = mybir.dt.int32
    u32 = mybir.dt.uint32
    Alu = mybir.AluOpType
    Act = mybir.ActivationFunctionType

    nc = bacc.Bacc(
        "TRN2",
        target_bir_lowering=False,
        debug=False,
        enable_asserts=False,
        num_devices=NC_CORES,
    )

    f_dr = nc.dram_tensor("features", [BPC, N, D], fp32, kind="ExternalInput").ap()
    ident_dr = nc.dram_tensor("identity", [128, 128], fp32, kind="ExternalInput").ap()
    rowb_dr = nc.dram_tensor("rowbase", [BPC, 1], fp32, kind="ExternalInput").ap()
    out_dr = nc.dram_tensor("slots", [BPC, NS, D], fp32, kind="ExternalOutput").ap()
    g_dr = nc.dram_tensor("g_scratch", [BPC * N, N], fp32, kind="Internal").ap()
    if debug:
        dbg_sal_dr = nc.dram_tensor("dbg_sal", [BPC, N], fp32, kind="ExternalOutput").ap()
        dbg_g_dr = nc.dram_tensor("dbg_g", [128, N], fp32, kind="ExternalOutput").ap()
        dbg_idx_dr = nc.dram_tensor("dbg_idx", [BPC, NS], fp32, kind="ExternalOutput").ap()
        dbg_sim_dr = nc.dram_tensor("dbg_sim", [BPC, N], fp32, kind="ExternalOutput").ap()

    with tile.TileContext(nc) as tc:
        with (
            tc.tile_pool(name="main", bufs=1) as mp,
            tc.tile_pool(name="fbuf", bufs=2) as fbp,
            tc.tile_pool(name="fnt", bufs=1) as ftp,
            tc.tile_pool(name="gst", bufs=4) as gsp,
            tc.tile_pool(name="small", bufs=2) as smp,
            tc.tile_pool(name="psA", bufs=2, space="PSUM") as ppA,
            tc.tile_pool(name="psB", bufs=2, space="PSUM") as ppB,
        ):
            ident = mp.tile([128, 128], fp32)
            nc.sync.dma_start(ident, ident_dr)
            rowb = mp.tile([BPC, 1], fp32)
            nc.sync.dma_start(rowb, rowb_dr)

            # persistent across phases
            sal_loop = mp.tile([BPC, N], fp32)             # saliency, loop layout
            wT = mp.tile([128, KT, BPC, NS], fp32)         # slot weights, lhsT layout
            wsum = mp.tile([BPC, NS], fp32)

            # ---------------- Phase A: per-batch normalize + Gram ----------
            for b in range(BPC):
                f_sb = fbp.tile([128, KT, D], fp32, tag="f")
                nc.sync.dma_start(
                    f_sb, f_dr[b].rearrange("(kt p) d -> p kt d", p=128)
                )
                sal2 = smp.tile([128, KT], fp32, tag="sal2")
                sq_scr = smp.tile([128, D], fp32, tag="sqscr")
                for kt in range(KT):
                    nc.scalar.activation(
                        sq_scr, f_sb[:, kt], Act.Square,
                        accum_out=sal2[:, kt:kt + 1],
                    )
                salb = smp.tile([128, KT], fp32, tag="salb")
                nc.scalar.activation(salb, sal2, Act.Sqrt)
                invb = smp.tile([128, KT], fp32, tag="invb")
                nc.vector.reciprocal(invb, salb)

                # saliency into loop layout [1, N] via PE transpose
                salT_ps = ppB.tile([KT, 128], fp32, tag="tps")
                nc.tensor.transpose(salT_ps, salb, ident)
                salT = smp.tile([KT, 128], fp32, tag="salT")
                nc.scalar.copy(salT, salT_ps)
                nc.sync.dma_start(sal_loop[b:b + 1, :], salT[:, :])

                # fn (bf16 copy for slot matmuls) then scale f in place -> fn32
                for kt in range(KT):
                    nc.vector.tensor_scalar(
                        f_sb[:, kt], f_sb[:, kt], invb[:, kt:kt + 1], None,
                        op0=Alu.mult,
                    )

                # transpose fn -> fnT [128(d), DT, N]
                fnT = ftp.tile([128, DT, N], fp32, tag="fnT")
                for kt in range(KT):
                    for dt in range(DT):
                        tp = ppB.tile([128, 128], fp32, tag="tps")
                        nc.tensor.transpose(
                            tp, f_sb[:, kt, dt * 128:(dt + 1) * 128], ident
                        )
                        if (kt + dt) % 2 == 0:
                            nc.scalar.copy(
                                fnT[:, dt, kt * 128:(kt + 1) * 128], tp
                            )
                        else:
                            nc.vector.tensor_copy(
                                fnT[:, dt, kt * 128:(kt + 1) * 128], tp
                            )

                # G = fnT.T @ fnT  (normalized Gram), row tiles -> DRAM
                for i in range(KT):
                    gps = ppA.tile([128, N], fp32, tag="gps")
                    for h in range(2):
                        for dt in range(DT):
                            nc.tensor.matmul(
                                gps[:, h * 512:(h + 1) * 512],
                                fnT[:, dt, i * 128:(i + 1) * 128],
                                fnT[:, dt, h * 512:(h + 1) * 512],
                                start=(dt == 0),
                                stop=(dt == DT - 1),
                            )
                    gstage = gsp.tile([128, N], fp32, tag="gstage")
                    nc.vector.tensor_copy(gstage[:, :512], gps[:, :512])
                    nc.scalar.copy(gstage[:, 512:], gps[:, 512:])
                    nc.sync.dma_start(
                        g_dr[b * N + i * 128: b * N + (i + 1) * 128, :], gstage
                    )

            # make sure all Gram writes to DRAM are visible before gathers
            tc.strict_bb_all_engine_barrier()

            if debug:
                dbg_gt = mp.tile([128, N], fp32)
                nc.sync.dma_start(dbg_gt, g_dr[0:128, :])
                nc.sync.dma_start(dbg_g_dr, dbg_gt)
                nc.sync.dma_start(dbg_sal_dr, sal_loop)
                dbg_idx_t = mp.tile([BPC, NS], fp32)

            # ---------------- Phase B: 16-step greedy loop -----------------
            mask = mp.tile([BPC, N], fp32)
            nc.vector.memset(mask, 1.0)
            msal = mp.tile([BPC, N], fp32)
            sim = mp.tile([BPC, N], fp32)
            mx8 = mp.tile([BPC, 8], fp32)
            idx8 = mp.tile([BPC, 8], u32)
            idxf = mp.tile([BPC, 1], fp32)
            rowidx = mp.tile([BPC, 1], i32)
            w1 = mp.tile([BPC, N], fp32)
            gate = mp.tile([BPC, N], fp32)
            aggw = mp.tile([BPC, N], fp32)
            aggw_bf = mp.tile([BPC, N], bf16)
            clipv = mp.tile([BPC, N], fp32)

            sim2 = mp.tile([BPC, N], fp32)
            w1b = mp.tile([BPC, N], fp32)
            sims = [sim, sim2]
            w1s = [w1, w1b]

            def emit_deferred(t):
                # off-critical aggregation work for step t (fills gather wait)
                s = sims[t % 2]
                w = w1s[t % 2]
                nc.vector.tensor_scalar(
                    gate, s, 0.5, None, op0=Alu.is_gt
                )
                nc.vector.tensor_mul(aggw, w, gate)
                nc.scalar.activation(
                    aggw_bf, aggw, Act.Copy,
                    accum_out=wsum[:, t:t + 1],
                )
                for kt in range(KT):
                    tp2 = ppB.tile([128, 128], fp32, tag="tps")
                    nc.tensor.transpose(
                        tp2[:, :BPC],
                        aggw[:, kt * 128:(kt + 1) * 128],
                        ident[:BPC, :BPC],
                    )
                    nc.scalar.copy(wT[:, kt, :, t], tp2[:, :BPC])

            for t in range(NS):
                s = sims[t % 2]
                nc.vector.tensor_mul(msal, sal_loop, mask)
                nc.vector.max(out=mx8, in_=msal)
                nc.vector.max_index(out=idx8, in_max=mx8, in_values=msal)
                nc.vector.tensor_copy(idxf, idx8[:, 0:1])
                nc.vector.tensor_scalar(
                    rowidx, idxf, rowb, None, op0=Alu.add
                )
                if debug:
                    nc.vector.tensor_copy(dbg_idx_t[:, t:t + 1], rowidx)
                nc.gpsimd.indirect_dma_start(
                    out=s,
                    out_offset=None,
                    in_=g_dr,
                    in_offset=IndirectOffsetOnAxis(ap=rowidx, axis=0),
                )
                if t > 0:
                    emit_deferred(t - 1)
                # critical tail: uses gathered sim
                nc.vector.tensor_mul(w1s[t % 2], s, mask)
                nc.vector.tensor_scalar(
                    clipv, s, 0.0, 1.0, op0=Alu.max, op1=Alu.min
                )
                nc.vector.tensor_scalar(
                    clipv, clipv, -1.0, 1.0, op0=Alu.mult, op1=Alu.add
                )
                nc.vector.tensor_mul(mask, mask, clipv)
            emit_deferred(NS - 1)

            # ---------------- Phase C: slot matmuls ------------------------
            nc.vector.tensor_scalar(wsum, wsum, 1e-8, None, op0=Alu.add)
            recip = mp.tile([BPC, NS], fp32)
            nc.vector.reciprocal(recip, wsum)
            rT_ps = ppB.tile([128, 128], fp32, tag="tps")
            nc.tensor.transpose(rT_ps[:NS, :BPC], recip, ident[:BPC, :BPC])
            recipT = mp.tile([NS, BPC], fp32)
            nc.scalar.copy(recipT, rT_ps[:NS, :BPC])

            for b in range(BPC):
                f_c = fbp.tile([128, KT, D], fp32, tag="f")
                nc.sync.dma_start(
                    f_c, f_dr[b].rearrange("(kt p) d -> p kt d", p=128)
                )
                sp = ppA.tile([NS, D], fp32, tag="gps")
                for h, (h0, h1) in enumerate([(0, 512), (512, D)]):
                    for kt in range(KT):
                        nc.tensor.matmul(
                            sp[:, h0:h1],
                            wT[:, kt, b, :],
                            f_c[:, kt, h0:h1],
                            start=(kt == 0),
                            stop=(kt == KT - 1),
                        )
                slot_sb = gsp.tile([NS, D], fp32, tag="slot")
                nc.scalar.activation(
                    slot_sb, sp, Act.Copy, scale=recipT[:, b:b + 1]
                )
                nc.sync.dma_start(out_dr[b], slot_sb)

    nc.compile()
    return nc


def _get_nc(debug=False):
    key = ("nc", debug)
    if key not in _CACHE:
        _CACHE[key] = _build_nc(debug)
    return _CACHE[key]


def kernel(features, batch_size=None, **_kw):
    from concourse import bass_utils

    nc = _get_nc()
    feats = np.ascontiguousarray(np.asarray(features, dtype=np.float32))
    ident = np.eye(128, dtype=np.float32)
    rowb = (np.arange(BPC, dtype=np.float32) * N).reshape(BPC, 1)
    in_maps = [
        {
            "features": feats[i * BPC:(i + 1) * BPC],
            "identity": ident,
            "rowbase": rowb,
        }
        for i in range(NC_CORES)
    ]
    res = bass_utils.run_bass_kernel_spmd(
        nc, in_maps, core_ids=list(range(NC_CORES))
    )
    outs = [np.asarray(res.results[i]["slots"]) for i in range(NC_CORES)]
    return np.concatenate(outs, axis=0).astype(np.float32)

